# revision 38
# baseline (speedup 1.0000x reference)
"""ADMM deconvolution (DiffuserCam-style) Trainium2 kernel, pair-split SPMD.

kernel(**inputs) takes FULL inputs, returns FULL output [1,3,486,648].

Parallelization: 2 cores per RGB channel (pairs (0,1),(2,3),(4,5); cores 6,7
idle; only contiguous replica groups are supported by the runtime). Within a
pair, core A (even) carries the {sk, a3, p1, p2} ADMM states and effectively
computes skp = ifft2(T); core B (odd) carries {vkp, a1, vd} and computes
Hskp = ifft2(H*T). All cores run the IDENTICAL program (SPMD, no branching);
the role difference is expressed purely through per-core input data:
  sel  [128,2]: plane0 = 1 on A cores, plane1 = 1 on B cores
  x    forward-transform input: A: s_grid (TV prox output), B: vd
  pw   freq-domain constants C1 = Smult*Heff, C2 = Smult*Heff*conj(H)
       (Heff = 1 on A, H on B; computed HOST-side with numpy fft2), so the
       single inverse transform of zhat = C1*S + m1*C2*V yields skp on A
       and Hskp on B.
Per iteration each core does ONE forward fft2 (of its local x) and ONE
inverse fft2 (of zhat), i.e. half the baseline transform work. The two
spectra are exchanged with a pairwise AllGather (2 l-chunks; gather slot 0 =
A's S_hat, slot 1 = B's V_hat, rank order). State updates (pass7) and the
TV prox (pass2) run on every core; garbage-side states are bounded and never
cross back into the real data path. The final per-image max is masked by
selA before the global AllReduce(max).

FFT2 as TensorEngine matmuls:
  stageA  height-rDFT [972 contracted over 8 row tiles, 488 bins]
  stageB  width-DFT   [1296 contracted per 128-col l-chunk]
  Binv    inverse width-DFT per 432-col chunk
  Ainv    inverse height-rDFT (ck-weighted); ifft scale folded into C1/C2.

Collective-dependent loads (gathered spectra) are issued from the ACT queue
so the SP DMA stream never head-of-line blocks on the AllGather; pass2's
DMA traffic is spread across the SP and ACT HWDGE queues.
"""

import sys

sys.path.insert(0, "/opt/trn_rl_repo")

import os
import numpy as np

D0, D1 = 486, 648
P0, P1 = D0 // 2, D1 // 2
HG, WG = 2 * D0, 2 * D1  # 972 x 1296
KH = HG // 2 + 1  # 487
KHP = KH + 1  # 488 (even; col 487 always zero)
K2 = 2 * KHP
ITERS = int(os.environ.get("K_ITERS", "10"))

NT = [(i * 128, min(128, HG - i * 128)) for i in range((HG + 127) // 128)]  # 8
WT = [(i * 128, min(128, WG - i * 128)) for i in range((WG + 127) // 128)]  # 11
KC = [(i * 122, 122) for i in range(4)]  # uniform k-chunks (488 = 4*122)
WC3 = [(i * 432, 432) for i in range(3)]
SCALE = float(HG * WG)
CROP = [(P0 + i * 128, min(128, D0 - i * 128)) for i in range((D0 + 127) // 128)]  # 4

N_XH_CHUNKS = int(os.environ.get("K_XH_CHUNKS", "2"))
XH_SPLIT = [(0, 6), (6, 11)] if N_XH_CHUNKS == 2 else [(0, 11)]


def _make_consts():
    n = np.arange(HG)
    k = np.arange(KH)
    w = np.arange(WG)
    ang_h = 2 * np.pi * np.outer(n, k) / HG
    fhr = np.zeros((HG, KHP), np.float32)
    fhi = np.zeros((HG, KHP), np.float32)
    fhr[:, :KH] = np.cos(ang_h)
    fhi[:, :KH] = -np.sin(ang_h)
    ang_w = 2 * np.pi * np.outer(w, w) / WG
    cw = np.cos(ang_w).astype(np.float32)
    sp = np.sin(ang_w).astype(np.float32)
    sn = (-sp).astype(np.float32)
    ck = np.full(KH, 2.0)
    ck[0] = 1.0
    ck[KH - 1] = 1.0
    ang_a = 2 * np.pi * np.outer(k, n) / HG
    atr = np.zeros((KHP, HG), np.float32)
    ati = np.zeros((KHP, HG), np.float32)
    atr[:KH] = ck[:, None] * np.cos(ang_a)
    ati[:KH] = -ck[:, None] * np.sin(ang_a)
    lapl = np.zeros((HG, WG), np.float32)
    lapl[0, 0] = 4.0
    lapl[0, 1] = lapl[1, 0] = lapl[0, -1] = lapl[-1, 0] = -1.0
    ltl = np.abs(np.fft.fft2(lapl)).astype(np.float32)  # [HG, WG]
    return fhr, fhi, cw, sp, sn, atr, ati, ltl


def _build(scalars):
    """scalars: list of (m1, m2, m3, tau) python floats, one per iteration."""
    import concourse.mybir as mybir
    from concourse import bacc, bass_isa
    from concourse.tile import TileContext

    F32 = mybir.dt.float32
    AF = mybir.ActivationFunctionType
    OP = mybir.AluOpType
    n_iters = len(scalars)

    sm_keys, sm_idx, vm_keys, vm_idx = [], [], [], []
    for m1, m2, m3, _ in scalars:
        key = (m1, m2, m3)
        if key not in sm_keys:
            sm_keys.append(key)
        sm_idx.append(sm_keys.index(key))
        if m1 not in vm_keys:
            vm_keys.append(m1)
        vm_idx.append(vm_keys.index(m1))

    nc = bacc.Bacc(None, num_devices=8)

    # register const APs for activation biases (ACT needs [128,1] AP biases)
    cvals = {1.0, -1.0}
    for m1, m2, m3, tau in scalars:
        cvals.update((-tau, tau))
    for v in sorted(cvals):
        if (mybir.dt.float32, v) not in nc.const_aps.aps:
            t = nc.alloc_sbuf_tensor(f"cst-{v!r}", [128, 1], mybir.dt.float32)
            nc.gpsimd.memset(t.ap(), v)
            nc.const_aps.aps[(mybir.dt.float32, v)] = t.ap()
    nc.all_engine_barrier()

    MDT = mybir.dt.float32r if os.environ.get("K_MM_DT", "f32r") == "f32r" else F32
    FDT = mybir.dt.bfloat16 if os.environ.get("K_FREQ_BF16", "1") == "1" else MDT
    CDT = mybir.dt.bfloat16 if os.environ.get("K_XH_DT", "bf16") == "bf16" else F32

    GROUPS = [[0, 1], [2, 3], [4, 5], [6, 7]]

    cty = nc.dram_tensor("cty", [HG, WG], F32, kind="ExternalInput")
    # per-row Vmult scale, one column per row-tile: vin inside sensor rows,
    # vout outside (Vmult = 1/(CtC+m1) is piecewise constant on the 2D grid)
    vsc = nc.dram_tensor("vsc", [len(vm_keys), 128, 8], F32, kind="ExternalInput")
    fhr = nc.dram_tensor("fhr", [HG, KHP], MDT, kind="ExternalInput")
    fhi = nc.dram_tensor("fhi", [HG, KHP], MDT, kind="ExternalInput")
    mats = nc.dram_tensor("mats", [3, WG, WG], FDT, kind="ExternalInput")
    atm = nc.dram_tensor("atm", [2, KHP, HG], MDT, kind="ExternalInput")
    PWDT = mybir.dt.float16 if os.environ.get("K_PW_F16", "1") == "1" else F32
    pw = nc.dram_tensor(
        "pw", [len(sm_keys), WG, 4, KHP], PWDT, kind="ExternalInput"
    )  # planes c1r, c1i, c2r, c2i; fp16: values <= ~8e-3, no overflow
    sel = nc.dram_tensor("sel", [128, 2], F32, kind="ExternalInput")
    out = nc.dram_tensor("out", [D0, D1], F32, kind="ExternalOutput")

    def mm(ps, lhsT, rhs, start, stop):
        nc.tensor.matmul(ps, lhsT, rhs, start=start, stop=stop)

    with TileContext(nc, pool_alloc_mode="queue") as tc:
        dram = tc.alloc_tile_pool(name="dstate", bufs=1, space="DRAM")
        st_d = [
            dram.tile([2, HG, WG], F32, name=f"st{i}") for i in range(n_iters + 1)
        ]  # planes: 0=a3, 1=qq (= m3*wkp - a3, computed by pass2)
        # a1/vkp eliminated (a1u = m1*(z-vd)); vd and sk live in SBUF only.
        # Last iteration stores its z (final skp) into st_d[n_iters][0].
        p1_d = [dram.tile([HG + 1, WG], F32, name=f"p1_{i}") for i in range(2)]
        p2_d = [dram.tile([HG, WG + 1], F32, name=f"p2_{i}") for i in range(2)]
        xh_d = [dram.tile([WG, K2], CDT, name=f"xh{i}") for i in range(2)]
        # per-chunk contiguous AllGather outputs: [2(rank), rows, K2]
        _chunk_rows = [
            (WT[c0][0], WT[c1 - 1][0] + WT[c1 - 1][1]) for c0, c1 in XH_SPLIT
        ]
        gh_d = [
            [
                dram.tile([2, l1 - l0, K2], CDT, name=f"gh{i}_{j}")
                for j, (l0, l1) in enumerate(_chunk_rows)
            ]
            for i in range(2)
        ]

        def gh_for_lc(it, lc):
            """(gather tile, row offset) holding l-chunk lc."""
            for j, (c0, c1) in enumerate(XH_SPLIT):
                if c0 <= lc < c1:
                    return gh_d[it % 2][j], WT[lc][0] - WT[c0][0]
            raise AssertionError

        ccin = dram.tile([1, 1], F32, name="ccin")
        ccout = dram.tile([1, 1], F32, name="ccout", addr_space="Shared")

        selp = tc.alloc_tile_pool(name="selp", bufs=1)
        selt = selp.tile([128, 2], F32, name="selt")
        nc.sync.dma_start(out=selt[:], in_=sel[:])
        vsct = selp.tile([128, len(vm_keys), 8], F32, name="vsct")
        for u in range(len(vm_keys)):
            nc.sync.dma_start(out=vsct[:, u], in_=vsc[u])

        # sensor region on the padded grid: rows [P0,P0+D0), cols [P1,P1+D1)
        def vmult_apply(vkn_ap, w_ap, p, nt, c0, csz, vmi, m1v):
            """vkn = Vmult*w on a [p, csz] chunk at grid cols c0..c0+csz."""
            vout = 1.0 / m1v
            cl = max(P1 - c0, 0)
            ch = min(P1 + D1 - c0, csz)
            if ch <= cl:
                nc.scalar.mul(vkn_ap, w_ap, vout)
                return
            if cl > 0:
                nc.scalar.mul(vkn_ap[:, 0:cl], w_ap[:, 0:cl], vout)
            nc.scalar.activation(
                vkn_ap[:, cl:ch], w_ap[:, cl:ch], AF.Copy, bias=0.0,
                scale=vsct[:p, vmi, nt : nt + 1],
            )
            if ch < csz:
                nc.scalar.mul(vkn_ap[:, ch:csz], w_ap[:, ch:csz], vout)

        big = tc.alloc_tile_pool(name="big", bufs=1)
        _slot_n = [0]

        def big_tile(shape, slot, dt=None):
            _slot_n[0] += 1
            return big.tile(
                shape, dt or MDT, name=f"bt{_slot_n[0]}", tag=slot, bufs=1
            )

        # ---------------- emitters ----------------

        def load_fh(fp):
            """[128, 8, 2, KHP] resident copy of fhr/fhi for a whole iteration"""
            fht = fp.tile([128, 8, 2, KHP], MDT, name="fhres")
            nc.sync.dma_start(
                out=fht[:, 0:7, 0],
                in_=fhr[:896].rearrange("(t p) k -> p t k", p=128),
            )
            nc.sync.dma_start(out=fht[:76, 7, 0], in_=fhr[896:])
            nc.sync.dma_start(
                out=fht[:, 0:7, 1],
                in_=fhi[:896].rearrange("(t p) k -> p t k", p=128),
            )
            nc.sync.dma_start(out=fht[:76, 7, 1], in_=fhi[896:])
            return fht

        def stageA(grid, ct, fht):
            """forward height-rDFT: grid [128,8,1296] -> ct [128,11,K2]"""
            for g0 in range(0, 11, 4):
                grp = [wc for wc in range(g0, min(g0 + 4, 11))]
                with tc.tile_pool(name="psa", bufs=1, space="PSUM") as pp:
                    pss = {}
                    for wc in grp:
                        for pl in range(2):
                            pss[(wc, pl)] = pp.tile(
                                [128, KHP], F32, name=f"ps_{wc}_{pl}"
                            )
                    for nt, (r0, p) in enumerate(NT):
                        for wc in grp:
                            w0, wsz = WT[wc]
                            for pl in range(2):
                                mm(
                                    pss[(wc, pl)][:wsz],
                                    grid[:p, nt, w0 : w0 + wsz],
                                    fht[:p, nt, pl],
                                    start=(nt == 0),
                                    stop=(nt == len(NT) - 1),
                                )
                    for wc in grp:
                        w0, wsz = WT[wc]
                        for pl in range(2):
                            nc.scalar.copy(
                                ct[:wsz, wc, pl * KHP : (pl + 1) * KHP],
                                pss[(wc, pl)][:wsz],
                            )

        def stageB(ct, sink):
            """width-DFT of ct; sink(lc, lsz, psr, psi) per l-chunk."""
            with (
                tc.tile_pool(name="cbp", bufs=2) as cbp,
                tc.tile_pool(name="psb", bufs=2, space="PSUM") as pp,
            ):
                for lc in range(11):
                    l0, lsz = WT[lc]
                    cb = cbp.tile([128, 3, 11, 128], FDT, name="cb", tag="cb")
                    for mi in range(3):
                        bulk = mats[mi, :1280, l0 : l0 + lsz].rearrange(
                            "(wt p) j -> p wt j", p=128
                        )
                        nc.sync.dma_start(out=cb[:, mi, :10, :lsz], in_=bulk)
                        nc.sync.dma_start(
                            out=cb[:16, mi, 10, :lsz],
                            in_=mats[mi, 1280:, l0 : l0 + lsz],
                        )
                    psr = pp.tile([128, KHP], F32, name="psb_r", tag="psbt_r")
                    psi = pp.tile([128, KHP], F32, name="psb_i", tag="psbt_i")
                    for wt in range(11):
                        w0, ksz = WT[wt]
                        # Yr += cw.T@Cr + sp.T@Ci ; Yi += sn.T@Cr + cw.T@Ci
                        for ps_, (ma, mb) in ((psr, (0, 1)), (psi, (2, 0))):
                            mm(
                                ps_[:lsz],
                                cb[:ksz, ma, wt, :lsz],
                                ct[:ksz, wt, 0:KHP],
                                start=(wt == 0),
                                stop=False,
                            )
                            mm(
                                ps_[:lsz],
                                cb[:ksz, mb, wt, :lsz],
                                ct[:ksz, wt, KHP:K2],
                                start=False,
                                stop=(wt == 10),
                            )
                    sink(lc, lsz, psr, psi)

        def xh_sink(it):
            """stageB sink: write local spectrum chunk to xh_d[it%2]."""
            xh = xh_d[it % 2]

            def sink(lc, lsz, psr, psi):
                l0 = WT[lc][0]
                with tc.tile_pool(name="xhp", bufs=2) as hp:
                    ht = hp.tile([128, K2], CDT, name="ht", tag="ht")
                    nc.scalar.copy(ht[:lsz, 0:KHP], psr[:lsz])
                    nc.scalar.copy(ht[:lsz, KHP:K2], psi[:lsz])
                    nc.sync.dma_start(out=xh[l0 : l0 + lsz], in_=ht[:lsz])

            return sink

        def exchange(it):
            """pairwise AllGather of xh_d[it%2] -> gh_d[it%2], chunked by l."""
            xh = xh_d[it % 2]
            for j, (l0, l1) in enumerate(_chunk_rows):
                nc.gpsimd.collective_compute(
                    "AllGather",
                    mybir.AluOpType.bypass,
                    replica_groups=GROUPS,
                    ins=[xh[l0:l1]],
                    outs=[gh_d[it % 2][j][:]],
                )

        def pointwise_zhat(it, zhat):
            """zhat = C1*S + m1*C2*V per l-chunk (C1/C2 host-precomputed)."""
            m1 = scalars[it][0]
            smi = sm_idx[it]
            for lc in range(11):
                l0, lsz = WT[lc]
                gh, goff = gh_for_lc(it, lc)
                with (
                    tc.tile_pool(name="pwin", bufs=2) as pin,
                    tc.tile_pool(name="pww", bufs=2) as twp,
                ):
                    sv = pin.tile([128, 2, K2], CDT, name="sv", tag="sv")
                    # issued from ACT so a pending AllGather never blocks
                    # the SP DMA stream (rv/atb prefetches behind it)
                    nc.scalar.dma_start(
                        out=sv[:lsz],
                        in_=gh[:, goff : goff + lsz].rearrange("m p k -> p m k"),
                    )
                    cc = pin.tile([128, 4, KHP], PWDT, name="cc", tag="cc")
                    # ACT-issued: keeps SP free for Binv rv/atb prefetches
                    nc.scalar.dma_start(out=cc[:lsz], in_=pw[smi, l0 : l0 + lsz])
                    c1r = cc[:lsz, 0]
                    c1i = cc[:lsz, 1]
                    c2r = cc[:lsz, 2]
                    c2i = cc[:lsz, 3]
                    sr = sv[:lsz, 0, 0:KHP]
                    si = sv[:lsz, 0, KHP:K2]
                    vr = sv[:lsz, 1, 0:KHP]
                    vi = sv[:lsz, 1, KHP:K2]
                    tw = twp.tile([128, 4, KHP], F32, name="tw", tag="tw")
                    u1 = tw[:lsz, 0]
                    u2 = tw[:lsz, 1]
                    u3 = tw[:lsz, 2]
                    u4 = tw[:lsz, 3]
                    zr = zhat[:lsz, lc, 0:KHP]
                    zi = zhat[:lsz, lc, KHP:K2]
                    # zr = (c1r*sr - c1i*si) + m1*(c2r*vr - c2i*vi)
                    nc.vector.tensor_mul(u1, c1r, sr)
                    nc.gpsimd.tensor_mul(u2, c1i, si)
                    nc.vector.tensor_mul(u3, c2r, vr)
                    nc.gpsimd.tensor_mul(u4, c2i, vi)
                    nc.vector.tensor_sub(u1, u1, u2)
                    nc.gpsimd.tensor_tensor(u3, u3, u4, op=OP.subtract)
                    nc.vector.scalar_tensor_tensor(
                        zr, u3, m1, u1, op0=OP.mult, op1=OP.add
                    )
                    # zi = (c1r*si + c1i*sr) + m1*(c2r*vi + c2i*vr)
                    nc.vector.tensor_mul(u1, c1r, si)
                    nc.gpsimd.tensor_mul(u2, c1i, sr)
                    nc.vector.tensor_mul(u3, c2r, vi)
                    nc.gpsimd.tensor_mul(u4, c2i, vr)
                    nc.gpsimd.tensor_add(u1, u1, u2)
                    nc.vector.tensor_add(u3, u3, u4)
                    nc.vector.scalar_tensor_tensor(
                        zi, u3, m1, u1, op0=OP.mult, op1=OP.add
                    )

        def binv_ainv_pass7(it, zhat, vd_next, skg):
            """Per 432-col chunk: inverse width-DFT (Binv), inverse
            height-rDFT (Ainv) -> z, then the fused ADMM state updates
            (z doubles as skp on A cores and Hskp on B cores)."""
            m1, m2, m3, _ = scalars[it]
            last = it == n_iters - 1
            m1n = scalars[it + 1][0] if not last else None
            wcp = tc.alloc_tile_pool(name="wchunk", bufs=1)
            for wc, (w0, wsz) in enumerate(WC3):
                w_c = wcp.tile(
                    [128, 2, 4, 432], MDT, name=f"wc{it}_{wc}", tag="wc", bufs=1
                )  # [k-in-chunk, pl, kc, w]
                for kh in range(2):
                    kcs = [kh * 2, kh * 2 + 1]
                    with (
                        tc.tile_pool(name="rvp", bufs=2) as rvp,
                        tc.tile_pool(name="psi", bufs=1, space="PSUM") as pp,
                    ):
                        pss = {}
                        for kc in kcs:
                            for pl in range(2):
                                pss[(kc, pl)] = pp.tile(
                                    [128, 432], F32, name=f"psi{kc}_{pl}"
                                )
                        for lt in range(11):
                            l0, lsz = WT[lt]
                            rv = rvp.tile([128, 3, 432], FDT, name="rv", tag="rv")
                            nc.sync.dma_start(
                                out=rv[:lsz],
                                in_=mats[:, l0 : l0 + lsz, w0 : w0 + wsz].rearrange(
                                    "m p j -> p m j"
                                ),
                            )
                            for kc in kcs:
                                k0, ksz = KC[kc]
                                tr_r = zhat[:lsz, lt, k0 : k0 + ksz]
                                tr_i = zhat[:lsz, lt, KHP + k0 : KHP + k0 + ksz]
                                # Wr = Tr@cw + Ti@sn ; Wi = Tr@sp + Ti@cw
                                mm(pss[(kc, 0)][:ksz], tr_r, rv[:lsz, 0],
                                   start=(lt == 0), stop=False)
                                mm(pss[(kc, 0)][:ksz], tr_i, rv[:lsz, 2],
                                   start=False, stop=(lt == 10))
                                mm(pss[(kc, 1)][:ksz], tr_r, rv[:lsz, 1],
                                   start=(lt == 0), stop=False)
                                mm(pss[(kc, 1)][:ksz], tr_i, rv[:lsz, 0],
                                   start=False, stop=(lt == 10))
                        for kc in kcs:
                            k0, ksz = KC[kc]
                            for pl in range(2):
                                nc.scalar.copy(
                                    w_c[:ksz, pl, kc], pss[(kc, pl)][:ksz]
                                )
                # ---- Ainv + pass7 on this column chunk ----
                with (
                    tc.tile_pool(name="atp", bufs=2) as atp,
                    tc.tile_pool(name="p7", bufs=2) as p7p,
                    tc.tile_pool(name="p7s", bufs=2) as p7s,
                    tc.tile_pool(name="psv", bufs=2, space="PSUM") as pvp,
                ):
                    for nt, (r0, p) in enumerate(NT):
                        atb = atp.tile([122, 2, 4, 128], MDT, name="atb", tag="atb")
                        for mi in range(2):
                            nc.scalar.dma_start(
                                out=atb[:, mi, :, :p],
                                in_=atm[mi, :, r0 : r0 + p].rearrange(
                                    "(kc q) n -> q kc n", q=122
                                ),
                            )
                        # co planes: 0=a3u, 1=z(skp)
                        co = p7s.tile([128, 2, 432], F32, name="co", tag="co")
                        z = co[:p, 1]
                        ps = pvp.tile([128, 432], F32, name="psv", tag="psv")
                        first = True
                        for kc in range(4):
                            k0, ksz = KC[kc]
                            for pl in range(2):
                                mm(
                                    ps[:p],
                                    atb[:ksz, pl, kc, :p],
                                    w_c[:ksz, pl, kc],
                                    start=first,
                                    stop=(kc == 3 and pl == 1),
                                )
                                first = False
                        nc.scalar.copy(z, ps[:p])
                        r_sl = slice(r0, r0 + p)
                        c_sl = slice(w0, w0 + wsz)
                        if last:
                            nc.sync.dma_start(
                                out=st_d[it + 1][0, r_sl, c_sl], in_=z
                            )
                            continue
                        # z also into the resident SBUF sk grid (for pass2)
                        nc.scalar.copy(skg[:p, nt, w0 : w0 + wsz], ps[:p])
                        sa = p7p.tile([128, 2, 432], F32, name="sa", tag="sa")
                        qqt = sa[:p, 0]
                        ctyt = sa[:p, 1]
                        wt2 = p7p.tile([128, 3, 432], F32, name="wt2", tag="wt2")
                        wk = wt2[:p, 0]
                        dd = wt2[:p, 1]
                        vkn = wt2[:p, 2]
                        if it > 0:
                            nc.sync.dma_start(out=qqt, in_=st_d[it][1, r_sl, c_sl])
                        nc.sync.dma_start(out=ctyt, in_=cty[r_sl, c_sl])
                        vdo = vd_next[:p, nt, w0 : w0 + wsz].bitcast(F32)
                        # dd = z - vd_old  (a1u = m1*dd, never materialized)
                        nc.vector.tensor_sub(dd, z, vdo)
                        a3u = co[:p, 0]
                        # a3u = a3 + m3*(z - wkp) = m3*z - qq
                        if it > 0:
                            nc.vector.scalar_tensor_tensor(
                                a3u, z, m3, qqt, op0=OP.mult, op1=OP.subtract
                            )
                        else:
                            nc.scalar.mul(a3u, z, m3)
                        # vkp_next = Vmult_next*(m1n*z + m1*dd + cty)
                        nc.gpsimd.tensor_scalar_mul(wk, z, m1n)
                        nc.vector.scalar_tensor_tensor(
                            wk, dd, m1, wk, op0=OP.mult, op1=OP.add
                        )
                        nc.gpsimd.tensor_add(wk, wk, ctyt)
                        vmult_apply(vkn, wk, p, nt, w0, wsz, vm_idx[it + 1], m1n)
                        nc.sync.dma_start(
                            out=st_d[it + 1][0, r_sl, c_sl], in_=a3u
                        )
                        # vd_next = vkp_next - (m1/m1n)*dd, in place in the
                        # resident SBUF vd slot (read of vd_old was above)
                        nc.vector.scalar_tensor_tensor(
                            vd_next[:p, nt, w0 : w0 + wsz],
                            dd,
                            -m1 / m1n,
                            vkn,
                            op0=OP.mult,
                            op1=OP.add,
                        )
            wcp.release()

        def pass2(it, vd_sb, x_grid, skg):
            """TV prox for iteration `it`: reads the resident SBUF sk grid
            (skg) + p/a3 state, writes p-state, qq (= m3*wkp - a3) for
            pass7, and the blended x = selA*s_grid + selB*vd into x_grid."""
            _, m2, m3, tau = scalars[it]
            p_zero = it == 1
            pb_prev = (p1_d[(it - 1) % 2], p2_d[(it - 1) % 2])
            pb_cur = (p1_d[it % 2], p2_d[it % 2])
            W1 = WG - 1
            with tc.tile_pool(name="p2a", bufs=2) as pa_pool:
                for nt, (r0, p) in enumerate(NT):
                    pS = p if r0 + p < HG else p - 1
                    w = pa_pool.tile([128, 7, WG], F32, name="w", tag="w")
                    skT = skg[:p, nt, :]
                    skS = w[:pS, 0]
                    p1o = w[:pS, 1]
                    p2o = w[:p, 2, 0:W1]
                    # skS = sk rows r0+1..r0+1+pS via SBUF->SBUF shift
                    nc.sync.dma_start(
                        out=w[0 : min(pS, 127), 0], in_=skg[1 : 1 + min(pS, 127), nt, :]
                    )
                    if pS == 128:
                        nc.sync.dma_start(out=w[127:128, 0], in_=skg[0:1, nt + 1, :])
                    if not p_zero:
                        nc.sync.dma_start(out=p1o, in_=pb_prev[0][r0 + 1 : r0 + 1 + pS])
                        nc.sync.dma_start(out=p2o, in_=pb_prev[1][r0 : r0 + p, 1:WG])
                    # qq = m3*wkp - a3 for pass7 (planes 5/6 are free until
                    # r2/rec are written later in the chain)
                    a3q = w[:p, 5]
                    qv = w[:p, 6]
                    nc.scalar.dma_start(out=a3q, in_=st_d[it][0, r0 : r0 + p])
                    nc.vector.scalar_tensor_tensor(
                        qv, skg[:p, nt, :], m3, a3q, op0=OP.mult, op1=OP.add
                    )
                    nc.scalar.activation(qv, qv, AF.Relu)  # m3*wkp
                    nc.gpsimd.tensor_tensor(qv, qv, a3q, op=OP.subtract)
                    nc.scalar.dma_start(out=st_d[it][1, r0 : r0 + p], in_=qv)
                    L1 = w[:pS, 3]
                    L2 = w[:p, 4, 0:W1]
                    nc.vector.tensor_sub(L1, skT[:pS], skS)
                    nc.gpsimd.tensor_tensor(
                        L2, skT[:, 0:W1], skT[:, 1:WG], op=OP.subtract
                    )
                    t1 = w[:pS, 0]  # overwrites skS
                    r1 = w[:pS, 1]  # overwrites p1o
                    t2 = w[:p, 2, 0:W1]  # overwrites p2o
                    r2 = w[:p, 5, 0:W1]
                    if p_zero:
                        nc.scalar.mul(t1, L1, 2.0)
                        nc.vector.tensor_copy(r1, L1)
                        nc.gpsimd.tensor_copy(r2, L2)
                        nc.scalar.mul(t2, L2, 2.0)
                    else:
                        nc.vector.scalar_tensor_tensor(
                            t1, L1, 2.0, p1o, op0=OP.mult, op1=OP.add
                        )
                        nc.gpsimd.tensor_tensor(r1, t1, L1, op=OP.subtract)
                        nc.gpsimd.tensor_add(r2, L2, p2o)
                        nc.gpsimd.tensor_add(t2, r2, L2)
                    sq1 = w[:p, 3]  # overwrites L1
                    if pS < p:
                        nc.vector.memset(sq1, 0.0)
                    nc.scalar.square(sq1[:pS], t1)
                    sq2 = w[:p, 4]  # overwrites L2
                    nc.vector.memset(w[:p, 4, W1:WG], 0.0)
                    nc.scalar.square(sq2[:, 0:W1], t2)
                    nc.vector.tensor_add(sq1, sq1, sq2)  # msq
                    mg = w[:p, 4]  # overwrites sq2
                    nc.scalar.sqrt(mg, sq1)
                    mgt = w[:p, 3]  # overwrites msq
                    nc.scalar.activation(mgt, mg, AF.Relu, bias=-tau)
                    den = w[:p, 4]  # overwrites mg
                    nc.scalar.activation(den, mgt, AF.Identity, bias=tau)
                    rec = w[:p, 6]
                    nc.vector.reciprocal_approx_fast(out=rec, in_=den)
                    mmlt = w[:p, 4]  # overwrites den
                    nc.vector.tensor_mul(mmlt, mgt, rec)
                    tm = w[:p, 3]  # overwrites mgt
                    nc.vector.tensor_mul(tm[:pS], t1, mmlt[:pS])
                    nc.gpsimd.tensor_tensor(r1, r1, tm[:pS], op=OP.subtract)
                    nc.sync.dma_start(out=pb_cur[0][r0 + 1 : r0 + 1 + pS], in_=r1)
                    nc.gpsimd.tensor_mul(tm[:, 0:W1], t2, mmlt[:, 0:W1])
                    nc.vector.tensor_sub(r2, r2, tm[:, 0:W1])
                    nc.sync.dma_start(out=pb_cur[1][r0 : r0 + p, 1:WG], in_=r2)
            with (
                tc.tile_pool(name="p2b_in", bufs=2) as pin,
                tc.tile_pool(name="p2b_w", bufs=2) as pwp,
            ):
                for nt, (r0, p) in enumerate(NT):
                    w4 = pin.tile([128, 3, WG + 1], F32, name="w4", tag="w4")
                    pa = w4[:p, 0, 0:WG]
                    pb = w4[:p, 1, 0:WG]
                    p2r = w4[:p, 2]
                    nc.sync.dma_start(out=pa, in_=pb_cur[0][r0 : r0 + p])
                    nc.sync.dma_start(out=pb, in_=pb_cur[0][r0 + 1 : r0 + 1 + p])
                    nc.scalar.dma_start(out=p2r, in_=pb_cur[1][r0 : r0 + p])
                    qq = pin.tile([128, WG], F32, name="qq", tag="qq")
                    nc.scalar.dma_start(out=qq[:p], in_=st_d[it][1, r0 : r0 + p])
                    w5 = pwp.tile([128, 2, WG], F32, name="w5", tag="w5")
                    va = w5[:p, 0]
                    nc.vector.tensor_sub(va, pa, pb)
                    vb = w4[:p, 0, 0:WG]  # reuse pa slot
                    nc.gpsimd.tensor_tensor(
                        vb, p2r[:, 0:WG], p2r[:, 1 : WG + 1], op=OP.subtract
                    )
                    nc.vector.tensor_add(va, va, vb)  # ltv
                    # s = m2*ltv + qq, then x = selA*s + selB*vd
                    nc.vector.scalar_tensor_tensor(
                        va, va, m2, qq[:p], op0=OP.mult, op1=OP.add
                    )
                    sA = w5[:p, 1]
                    nc.scalar.activation(
                        sA, va, AF.Copy, bias=0.0, scale=selt[:p, 0:1]
                    )
                    vB = w4[:p, 1, 0:WG]  # reuse pb slot
                    nc.scalar.activation(
                        vB, vd_sb[:p, nt, :].bitcast(F32), AF.Copy, bias=0.0,
                        scale=selt[:p, 1:2],
                    )
                    nc.gpsimd.tensor_add(x_grid[:p, nt, :], sA, vB)

        # ================= program =================

        # --- prologue: p-buffer guard zeroing ---
        with tc.tile_pool(name="zg", bufs=1) as zp:
            zt = zp.tile([128, WG], F32, name="zt")
            nc.vector.memset(zt[:], 0.0)
            for b in range(2):
                nc.sync.dma_start(out=p1_d[b][0:1], in_=zt[0:1])
                nc.sync.dma_start(out=p1_d[b][HG : HG + 1], in_=zt[0:1])
                for nt, (r0, p) in enumerate(NT):
                    nc.sync.dma_start(out=p2_d[b][r0 : r0 + p, 0:1], in_=zt[:p, 0:1])
                    nc.sync.dma_start(
                        out=p2_d[b][r0 : r0 + p, WG : WG + 1], in_=zt[:p, 0:1]
                    )

        # --- prologue: vd_0 = vkp_0 = Vmult*Cty -> SBUF vd slot; x_0 = selB*vd_0 ---
        xg = big_tile([128, 8, WG], "xg")
        vd_slot = big_tile([128, 8, WG], "vd")
        with tc.tile_pool(name="v0in", bufs=2) as vip:
            for nt, (r0, p) in enumerate(NT):
                vin = vip.tile([128, 2, WG], F32, name="vin", tag="vin")
                nc.sync.dma_start(out=vin[:p, 0], in_=cty[r0 : r0 + p])
                vmult_apply(
                    vin[:p, 1], vin[:p, 0], p, nt, 0, WG, vm_idx[0], vm_keys[vm_idx[0]]
                )
                nc.gpsimd.tensor_copy(vd_slot[:p, nt, :], vin[:p, 1])
                nc.scalar.activation(
                    xg[:p, nt, :], vin[:p, 1], AF.Copy, bias=0.0,
                    scale=selt[:p, 1:2],
                )

        # --- iterations: slots A=xg, B=ct/vd, C=zhat ---
        for it in range(n_iters):
            last = it == n_iters - 1
            with tc.tile_pool(name="fhp", bufs=1) as fp:
                fh = load_fh(fp)
                ct = big_tile([128, 11, K2], "bb", dt=FDT)
                stageA(xg, ct, fh)  # xg dies
                stageB(ct, xh_sink(it))  # ct dies
            exchange(it)
            zhat = big_tile([128, 11, K2], "zz", dt=FDT)
            pointwise_zhat(it, zhat)
            skg = big_tile([128, 8, WG], "xg", dt=F32)  # xg bytes, now sk
            binv_ainv_pass7(it, zhat, vd_slot, skg)  # zhat dies
            if not last:
                xg = big_tile([128, 8, WG], "xg")
                pass2(it + 1, vd_slot, xg, skg)  # skg dies

        # --- epilogue: crop, masked global max, normalize ---
        with (
            tc.tile_pool(name="ep", bufs=1) as ep,
            tc.tile_pool(name="eps", bufs=1) as eps,
        ):
            mxs = eps.tile([128, 4], F32, name="mxs")
            nc.vector.memset(mxs[:], 0.0)
            ctiles = []
            for t, (r0, p) in enumerate(CROP):
                ctile = ep.tile([128, D1], F32, name=f"ctile{t}", bufs=1)
                nc.sync.dma_start(
                    out=ctile[:p], in_=st_d[n_iters][0, r0 : r0 + p, P1 : P1 + D1]
                )
                ctiles.append(ctile)
                nc.vector.tensor_reduce(
                    out=mxs[:p, t : t + 1],
                    in_=ctile[:p],
                    axis=mybir.AxisListType.X,
                    op=mybir.AluOpType.max,
                )
            mx1 = eps.tile([128, 1], F32, name="mx1")
            nc.vector.tensor_reduce(
                out=mx1[:], in_=mxs[:], axis=mybir.AxisListType.X,
                op=mybir.AluOpType.max,
            )
            # mask: only A cores contribute to the global max
            nc.scalar.activation(
                mx1[:], mx1[:], AF.Copy, bias=0.0, scale=selt[:, 0:1]
            )
            mxr = eps.tile([128, 1], F32, name="mxr")
            nc.gpsimd.partition_all_reduce(
                mxr[:], mx1[:], channels=128, reduce_op=bass_isa.ReduceOp.max
            )
            nc.sync.dma_start(out=ccin[:], in_=mxr[0:1])
            nc.gpsimd.collective_compute(
                "AllReduce",
                mybir.AluOpType.max,
                replica_groups=[[0, 1, 2, 3, 4, 5, 6, 7]],
                ins=[ccin[:]],
                outs=[ccout[:]],
            )
            gmx = eps.tile([128, 1], F32, name="gmx")
            nc.sync.dma_start(out=gmx[0:1], in_=ccout[:])
            gmxb = eps.tile([128, 1], F32, name="gmxb")
            nc.gpsimd.partition_broadcast(gmxb[:], gmx[0:1], channels=128)
            rcp = eps.tile([128, 1], F32, name="rcp")
            nc.vector.reciprocal(out=rcp[:], in_=gmxb[:])
            for t, (r0, p) in enumerate(CROP):
                o = ep.tile([128, D1], F32, name=f"o{t}", bufs=1)
                nc.scalar.activation(
                    o[:p], ctiles[t][:p], AF.Copy, bias=0.0, scale=rcp[:p]
                )
                nc.sync.dma_start(out=out[r0 - P0 : r0 - P0 + p], in_=o[:p])

        big.release()
        selp.release()

    nc.finalize()
    return nc


def _mats_np(cw, sp, sn):
    m = np.ascontiguousarray(np.stack([cw, sp, sn]))
    if os.environ.get("K_FREQ_BF16", "1") == "1":
        import ml_dtypes

        m = m.astype(ml_dtypes.bfloat16)
    return m


_BUILD_CACHE = {}
_CONSTS = None


def _cached_consts():
    global _CONSTS
    if _CONSTS is None:
        _CONSTS = _make_consts()
    return _CONSTS


def _prepare(y, h, mu1, mu2, mu3, tau):
    y = np.asarray(y, dtype=np.float32)
    h = np.asarray(h, dtype=np.float32)
    scalars = tuple(
        (float(mu1[i]), float(mu2[i]), float(mu3[i]), float(tau[i]))
        for i in range(ITERS)
    )
    if scalars not in _BUILD_CACHE:
        _BUILD_CACHE[scalars] = _build(list(scalars))
    nc = _BUILD_CACHE[scalars]

    fhr, fhi, cw, sp, sn, atr, ati, ltl = _cached_consts()

    sm_keys, vm_keys = [], []
    for m1, m2, m3, _ in scalars:
        if (m1, m2, m3) not in sm_keys:
            sm_keys.append((m1, m2, m3))
        if m1 not in vm_keys:
            vm_keys.append(m1)
    # per-row Vmult scales for the sensor-column range (piecewise constant)
    vscs = np.zeros((len(vm_keys), 128, 8), np.float32)
    for u, m1 in enumerate(vm_keys):
        for nt, (r0, p) in enumerate(NT):
            rows = r0 + np.arange(128)
            inside = (rows >= P0) & (rows < P0 + D0)
            vscs[u, :, nt] = np.where(inside, 1.0 / (1.0 + m1), 1.0 / m1)

    hpad = np.zeros((HG, WG), np.float32)
    hpad[P0 : P0 + D0, P1 : P1 + D1] = h
    hps = np.fft.ifftshift(hpad)
    # host-side FFT-domain constants (freq layout [l(w-dim), k(h-dim bins)])
    H = np.fft.fft2(hps)  # [HG, WG]
    Hlk = H[:KH, :].T  # [WG, KH]
    HtH_lk = (np.abs(H) ** 2)[:KH, :].T
    ltl_lk = ltl[:KH, :].T

    def pad_k(a):
        o = np.zeros((WG, KHP), a.dtype)
        o[:, :KH] = a
        return o

    pw_maps = {}  # role -> [nsm, WG, 4, KHP]
    for role in ("A", "B", "Z"):
        planes_all = []
        for m1, m2, m3 in sm_keys:
            sm = 1.0 / (SCALE * (m1 * HtH_lk + m2 * ltl_lk + m3))  # [WG,KH]
            if role == "A":
                c1 = sm.astype(np.complex64)
                c2 = sm * np.conj(Hlk)
            elif role == "B":
                c1 = sm * Hlk
                c2 = sm * HtH_lk
            else:
                c1 = np.zeros_like(Hlk)
                c2 = np.zeros_like(Hlk)
            planes = np.stack(
                [
                    pad_k(np.real(c1).astype(np.float32)),
                    pad_k(np.imag(c1).astype(np.float32)),
                    pad_k(np.real(c2).astype(np.float32)),
                    pad_k(np.imag(c2).astype(np.float32)),
                ],
                axis=1,
            )  # [WG, 4, KHP]
            planes_all.append(planes)
        pwa = np.ascontiguousarray(np.stack(planes_all))
        if os.environ.get("K_PW_F16", "1") == "1":
            pwa = pwa.astype(np.float16)
        pw_maps[role] = pwa

    common = {
        "vsc": vscs,
        "fhr": fhr,
        "fhi": fhi,
        "mats": _mats_np(cw, sp, sn),
        "atm": np.ascontiguousarray(np.stack([atr, ati])),
    }
    zero_cty = np.zeros((HG, WG), np.float32)
    ctys = []
    for c in range(3):
        ctyc = np.zeros((HG, WG), np.float32)
        ctyc[P0 : P0 + D0, P1 : P1 + D1] = y[0, c]
        ctys.append(ctyc)
    in_maps = []
    for c in range(8):
        m = dict(common)
        selv = np.zeros((128, 2), np.float32)
        if c < 6:
            m["cty"] = ctys[c // 2]
            selv[:, c % 2] = 1.0  # even cores A (sel plane 0), odd B
            m["pw"] = pw_maps["A" if c % 2 == 0 else "B"]
        else:
            m["cty"] = zero_cty
            m["pw"] = pw_maps["Z"]
        m["sel"] = selv
        in_maps.append(m)
    return nc, in_maps


def kernel(y, h, mu1, mu2, mu3, tau):
    from concourse.bass_utils import run_bass_kernel_spmd

    nc, in_maps = _prepare(y, h, mu1, mu2, mu3, tau)
    trace = os.environ.get("K_TRACE", "0") == "1"
    res = run_bass_kernel_spmd(nc, in_maps, core_ids=list(range(8)), trace=trace)
    kernel._exec_ns = res.exec_time_ns
    kernel._res = res
    outp = np.stack([res.results[2 * c]["out"] for c in range(3)])[None]
    return outp.astype(np.float32)
